# revision 92
# baseline (speedup 1.0000x reference)
"""Trainium2 Bass kernel for nn_ExpSelfAttention (dense transformer block).

Math (per batch item b):
    y  = LN(x; g1, beta1);  z = y @ w_lin.T + b_lin
    attn = W @ z            (W = causal exp-decay matrix, alpha=0.9)
    x2 = x + attn
    y2 = LN(x2; g2, beta2); h = relu(y2 @ w1.T + b1)
    out = x2 + h @ w2.T + b2

Sharding: data parallel over batch (16 / 8 cores = 2 per core); weights and
the (input-independent) decay-matrix blocks replicated. No collectives.

Kernel strategy per core (mixed precision):
  - Attention path in bf16 (proj + banded mixing matmuls, PE transposes at
    1 cyc/row); residuals and LN stats in f32. b_lin folded into the mixing
    PSUM via a K=1 ones-row bias matmul (W rows sum to 1, so W@(z+zb)=W@z+zb).
  - FFN matmuls in fp8e4 (e4m3) with DoubleRow perf mode: 256 contraction
    rows per instruction at 0.5 cyc per output element - 4x less PE time
    than f32r/bf16. Weights pre-scaled by 16 (w1) / 32 (w2) on the host to
    center fp8's [2^-6, 240] range; the 512x net factor is unwound in the
    output eviction (po * 1/512 + x2b).
  - The S x S decay matmul is block-banded (alpha^128 ~ 1.4e-6): exact
    block-diag + 1 lag matmul per 128-token block.
  - Engine balance: LN normalizes ((x-m)*rsqrt, subtract+mult - the Pool
    ucode has no divide) run on the otherwise-idle Pool (gpsimd) engine
    (SBUF-only ops - Pool has no PSUM port); PSUM evictions split between
    ACT and DVE; sqrt batched in pairs on ACT + DVE reciprocal. LN2
    transposes run in bf16 (hw fp8 transpose needs stride-2 outputs) and
    cast to fp8 at the ACT eviction.
  - FFN1 PSUM banks hold both 256-token half-chunk groups of one f-tile
    (sequential accumulation groups; PSUM data persists across a group
    start in the same bank), so each relu eviction covers [128,512].
  - Software pipelined in 512-token steps: step i+1's load/LN1/
    transpose/proj run between step i's mixing and FFN.
"""

import sys
from contextlib import ExitStack

for _p in ("/opt/trn_rl_repo", "/opt/pypackages"):
    if _p not in sys.path:
        sys.path.insert(0, _p)

import numpy as np
import ml_dtypes

import concourse.bass as bass
import concourse.mybir as mybir
import concourse.tile as tile
from concourse import bacc
from concourse.bass_utils import run_bass_kernel_spmd
from concourse.masks import make_identity

ALPHA, EPS = 0.9, 1e-5
S, B, D, FF = 2048, 16, 512, 2048
NCORES = 8
BL = B // NCORES            # batch items per core
T = 128                     # mixing block
CB = 4                      # blocks per chunk (step = 512 tokens)
NBLK = S // T               # 16
NCHUNK = NBLK // CB         # 4
NFT = FF // 128             # 16 f-tiles
KD = D // 128               # 4 d-tiles
NLAG = 1                    # decay lag blocks kept (lag>=2 < 2e-12 relative)
W1SC, W2SC = 16.0, 32.0     # fp8 weight pre-scales
OSC = 1.0 / (W1SC * W2SC)   # output unscale
RELU_DVE = frozenset({1, 3, 5, 7, 9, 11, 13, 15})  # relu evicts on DVE

F32 = mybir.dt.float32
BF16 = mybir.dt.bfloat16
F8 = mybir.dt.float8e4
AF = mybir.ActivationFunctionType
ALU = mybir.AluOpType
DR = mybir.MatmulPerfMode.DoubleRow

NPBF16 = ml_dtypes.bfloat16
NPF8 = mybir.dt.np(F8)      # ml_dtypes.float8_e4m3 (max 240)


def _host_consts():
    """Decay-matrix derived constants, f64 -> f32 (mirrors reference)."""
    i = np.arange(S, dtype=np.float64)
    diff = i[:, None] - i[None, :]
    with np.errstate(under="ignore"):
        W = np.where(diff >= 0, ALPHA ** (diff + 1), 0.0)
        W = W + np.diag(1.0 - W.sum(axis=1))
        W = W.astype(np.float32)
        blocks = [
            np.ascontiguousarray(W[c * T : (c + 1) * T, c * T : (c + 1) * T].T)
            for c in range(NBLK)
        ]
        uniq, idx = [], []
        for blk in blocks:
            for j, u in enumerate(uniq):
                if np.array_equal(blk, u):
                    idx.append(j)
                    break
            else:
                idx.append(len(uniq))
                uniq.append(blk)
        wblkT = np.stack(uniq)  # [NU, T, T]
        lags = []
        for l in range(1, NLAG + 1):
            L = W[l * T : (l + 1) * T, 0:T]
            lags.append(np.ascontiguousarray(L.T))
        wlagT = np.stack(lags)  # [NLAG, T, T]
    return wblkT.astype(np.float32), idx, wlagT.astype(np.float32)


_WBLKT, _BLKIDX, _WLAGT = _host_consts()
NU = _WBLKT.shape[0]

_NC_CACHE = {}


def build_nc():
    key = "v2"
    if key in _NC_CACHE:
        return _NC_CACHE[key]
    nc = bacc.Bacc()

    x_d = nc.declare_dram_parameter("x", [S, BL, D], F32, isOutput=False)
    wp_d = nc.declare_dram_parameter("wp", [D, D], BF16, isOutput=False)
    zb_d = nc.declare_dram_parameter("zb", [D], BF16, isOutput=False)
    w1_d = nc.declare_dram_parameter("w1t8", [D, FF], F8, isOutput=False)
    hb_d = nc.declare_dram_parameter("hb16", [FF], F32, isOutput=False)
    w2_d = nc.declare_dram_parameter("w2t8", [FF, D], F8, isOutput=False)
    b2_d = nc.declare_dram_parameter("b2", [D], F32, isOutput=False)
    wblk_d = nc.declare_dram_parameter("wblk", [NU, T, T], BF16, isOutput=False)
    wlag_d = nc.declare_dram_parameter("wlag", [NLAG, T, T], BF16, isOutput=False)
    out_d = nc.declare_dram_parameter("out", [S, BL, D], F32, isOutput=True)

    with tile.TileContext(nc) as tc, ExitStack() as ctx:
        pool = lambda name, bufs, **kw: ctx.enter_context(
            tc.tile_pool(name=name, bufs=bufs, **kw)
        )
        wgt = pool("wgt", 1)
        stage = pool("stage", 1)
        xin = pool("xin", 9)
        lnp = pool("ln", 4)
        yppp = pool("ypp", 7)
        xtp = pool("xt", 6)
        y2tp = pool("y2t", 2)
        zp = pool("z", 10)
        x2p = pool("x2", 5)
        x2bp = pool("x2b", 5)
        hp = pool("h", 2)
        outp = pool("outp", 3)
        psmm = pool("psmm", 5, space="PSUM")
        pstr = pool("pstr", 3, space="PSUM")

        # ---------------- one-time setup ----------------
        # Interleave the two batch items: consecutive steps are then fully
        # independent (the decay-lag z dependency is within one item), so
        # each step's LN chains overlap the neighbor step's FFN matmuls.
        steps = [(b, c) for c in range(NCHUNK) for b in range(BL)]
        xpre = {}

        def preload_x(i):
            if i >= len(steps) or i in xpre:
                return
            b, c = steps[i]
            tiles = []
            for t in range(CB):
                s0 = (c * CB + t) * T
                xt = xin.tile([128, D], F32, tag="x")
                nc.sync.dma_start(xt[:], x_d.ap()[s0 : s0 + T, b, :])
                tiles.append(xt)
            xpre[i] = tiles

        preload_x(0)
        preload_x(1)

        ident_f = stage.tile([128, 128], F32, tag="ident_f")
        make_identity(nc, ident_f[:])
        ident_bf = wgt.tile([128, 128], BF16, tag="ident_bf")
        nc.vector.tensor_copy(ident_bf[:], ident_f[:])
        ident_f8 = wgt.tile([128, 128], F8, tag="ident_f8")
        nc.vector.tensor_copy(ident_f8[:], ident_f[:])
        eps_t = wgt.tile([128, 1], F32, tag="eps")
        nc.vector.memset(eps_t[:], EPS)
        ones_r = wgt.tile([1, 128], BF16, tag="ones_r")
        nc.vector.memset(ones_r[:], 1.0)
        zb_r = wgt.tile([1, D], BF16, tag="zb_r")
        nc.sync.dma_start(zb_r[:], bass.AP(tensor=zb_d, offset=0, ap=[[0, 1], [1, D]]))
        b2_bc = wgt.tile([128, D], F32, tag="b2")
        nc.sync.dma_start(
            b2_bc[:], bass.AP(tensor=b2_d, offset=0, ap=[[0, 128], [1, D]])
        )
        hb_sb = wgt.tile([128, NFT], F32, tag="hb")
        nc.sync.dma_start(
            hb_sb[:], bass.AP(tensor=hb_d, offset=0, ap=[[1, 128], [128, NFT]])
        )
        wp_r = wgt.tile([128, KD, D], BF16, tag="wp")
        nc.sync.dma_start(wp_r[:], wp_d.ap().rearrange("(kd p) e -> p kd e", p=128))
        wblk_r = wgt.tile([128, NU, T], BF16, tag="wblk")
        nc.sync.dma_start(wblk_r[:], wblk_d.ap().rearrange("b j r -> j b r"))
        wlag_r = wgt.tile([128, NLAG, T], BF16, tag="wlag")
        nc.sync.dma_start(wlag_r[:], wlag_d.ap().rearrange("b j r -> j b r"))

        # ---------------- helpers ----------------
        def ln_stats(xts, tag):
            """4 tiles' LN stats -> (mvs [128,4,2], rs4 [128,4]=rsqrt(v+eps))."""
            mvs = lnp.tile([128, CB, 2], F32, tag=tag + "mv")
            rs4 = lnp.tile([128, CB], F32, tag=tag + "rs")
            for t in range(CB):
                st = lnp.tile([128, 6], F32, tag=tag + "st")
                nc.vector.bn_stats(st[:], xts[t][:])
                nc.vector.bn_aggr(mvs[:, t, :], st[:])
                if t % 2 == 1:
                    nc.scalar.activation(
                        rs4[:, t - 1 : t + 1],
                        mvs[:, t - 1 : t + 1, 1:2].rearrange("p a b -> p (a b)"),
                        AF.Sqrt, bias=eps_t[:], scale=1.0,
                    )
                    nc.vector.reciprocal(
                        rs4[:, t - 1 : t + 1], rs4[:, t - 1 : t + 1]
                    )
            return mvs, rs4

        def norm_transp(xt, mvs, sq4, t, odt, ident, dest_ap, src_pat):
            """(xt - m)/sq -> odt tile (Pool), PE-transpose, ACT-evict."""
            ypp = yppp.tile([128, D], odt, tag="ypp")
            nc.gpsimd.tensor_scalar(
                out=ypp[:], in0=xt[:],
                scalar1=mvs[:, t, 0:1], scalar2=sq4[:, t : t + 1],
                op0=ALU.subtract, op1=ALU.mult,
            )
            pt = pstr.tile([128, 512], odt, tag="tr")
            for kd in range(KD):
                nc.tensor.transpose(
                    pt[:, kd * 128 : (kd + 1) * 128],
                    ypp[:, kd * 128 : (kd + 1) * 128],
                    ident[:],
                )
            nc.scalar.activation(dest_ap, src_pat(pt[:]), AF.Copy)

        # ---------------- pipeline stages ----------------
        zall = {b: [] for b in range(BL)}
        a_out, b_out = {}, {}

        ypp_pre = {}

        def stage_a_pre(i):
            """LN1 stats + normalize for step i -> ypp tiles (bf16)."""
            if i >= len(steps):
                return
            preload_x(i)
            xts = xpre[i]
            mvs, sq4 = ln_stats(xts, "l1")
            ypps = []
            for t in range(CB):
                ypp = yppp.tile([128, D], BF16, tag="y1pp")
                nc.gpsimd.tensor_scalar(
                    out=ypp[:], in0=xts[t][:],
                    scalar1=mvs[:, t, 0:1], scalar2=sq4[:, t : t + 1],
                    op0=ALU.subtract, op1=ALU.mult,
                )
                ypps.append(ypp)
            ypp_pre[i] = ypps

        def stage_a_post(i):
            """PE transposes + ACT evicts of the normalized LN1 tiles."""
            if i >= len(steps):
                return
            xts, xT = xpre.pop(i), []
            for t, ypp in enumerate(ypp_pre.pop(i)):
                pt = pstr.tile([128, 512], BF16, tag="tr")
                for kd in range(KD):
                    nc.tensor.transpose(
                        pt[:, kd * 128 : (kd + 1) * 128],
                        ypp[:, kd * 128 : (kd + 1) * 128],
                        ident_bf[:],
                    )
                xTt = xtp.tile([128, KD, 128], BF16, tag="xT")
                nc.scalar.activation(
                    xTt[:], pt[:].rearrange("p (a b) -> p a b", b=128), AF.Copy
                )
                xT.append(xTt)
            a_out[i] = (xts, xT)

        def stage_b_mm(i):
            if i >= len(steps):
                return
            _, xT = a_out[i]
            pzs = []
            for t in range(CB):
                pz = psmm.tile([128, D], F32, tag="mm")
                for kd in range(KD):
                    nc.tensor.matmul(
                        pz[:], xT[t][:, kd, :], wp_r[:, kd, :],
                        start=(kd == 0), stop=(kd == KD - 1),
                    )
                pzs.append(pz)
            b_out[i] = pzs

        def stage_b_evict(i):
            if i >= len(steps):
                return
            b, c = steps[i]
            for t in range(CB):
                zt = zp.tile([128, D], BF16, tag="z")
                if t % 2 == 0:
                    nc.scalar.activation(zt[:], b_out[i][t][:], AF.Copy)
                else:
                    nc.vector.tensor_copy(zt[:], b_out[i][t][:])
                zall[b].append(zt)
            del b_out[i]

        stage_a_pre(0)
        stage_a_post(0)
        stage_b_mm(0)
        stage_b_evict(0)

        # big fp8 FFN weights: DMA'd in chunks interleaved with the early
        # pipeline so x loads and the first FFN aren't blocked.
        w18_r = wgt.tile([128, 2, 2, FF], F8, tag="w18")
        w28_r = wgt.tile([128, NFT // 2, 2, D], F8, tag="w28")
        w18_ap = w1_d.ap().rearrange("(kd2 i p) f -> p kd2 i f", p=128, i=2)
        w28_ap = w2_d.ap().rearrange("(kt i p) e -> p kt i e", p=128, i=2)
        wload = [
            lambda kd2=kd2: nc.sync.dma_start(
                w18_r[:, kd2, :, :], w18_ap[:, kd2, :, :]
            )
            for kd2 in range(2)
        ] + [
            lambda k4=k4: nc.sync.dma_start(
                w28_r[:, 2 * k4 : 2 * k4 + 2, :, :],
                w28_ap[:, 2 * k4 : 2 * k4 + 2, :, :],
            )
            for k4 in range(4)
        ]
        wload.reverse()
        for _ in range(2):
            wload.pop()()

        for i, (b, c) in enumerate(steps):
            xts, _ = a_out.pop(i)
            x2ts, x2bts, pms = [], [], []
            # --- mixing (banded) + zb ones-row bias matmul ---
            for t in range(CB):
                blk = c * CB + t
                nmix = 1 + min(blk, NLAG)
                pm = psmm.tile([128, D], F32, tag="mm")
                for l in range(nmix):
                    lhs = (
                        wblk_r[:, _BLKIDX[blk], :] if l == 0
                        else wlag_r[:, l - 1, :]
                    )
                    nc.tensor.matmul(
                        pm[:], lhs, zall[b][blk - l][:],
                        start=(l == 0), stop=False,
                    )
                nc.tensor.matmul(pm[:], ones_r[:], zb_r[:], start=False, stop=True)
                pms.append(pm)
            # --- next step's LN1 chain ---
            stage_a_pre(i + 1)
            stage_a_post(i + 1)
            for _ in range(min(2, len(wload))):
                wload.pop()()
            # --- x2 = pm + x (DVE, accumulating sum(x2) for free);
            # x2b = x2 + b2 (Pool); sum(x2^2) via ACT Square accumulator,
            # so LN2 statistics cost DVE only a few [128,4] ops ---
            sums = lnp.tile([128, CB], F32, tag="l2su")
            sqs = lnp.tile([128, CB], F32, tag="l2sq")
            mvs2 = lnp.tile([128, CB, 2], F32, tag="l2mv")
            q4 = lnp.tile([128, CB], F32, tag="l2q4")
            m2 = lnp.tile([128, CB], F32, tag="l2m2")
            var4 = lnp.tile([128, CB], F32, tag="l2v4")
            sq42 = lnp.tile([128, CB], F32, tag="l2rs")
            mean_v = mvs2[:, :, 0:1].rearrange("p a b -> p (a b)")
            for t in range(CB):
                x2t = x2p.tile([128, D], F32, tag="x2")
                nc.vector.scalar_tensor_tensor(
                    out=x2t[:], in0=pms[t][:], scalar=0.0, in1=xts[t][:],
                    op0=ALU.add, op1=ALU.add, accum_out=sums[:, t : t + 1],
                )
                x2ts.append(x2t)
                x2bt = x2bp.tile([128, D], F32, tag="x2b")
                nc.gpsimd.tensor_add(x2bt[:], x2t[:], b2_bc[:])
                x2bts.append(x2bt)
                scr = yppp.tile([128, D], BF16, tag="sqscr")
                nc.scalar.activation(
                    scr[:], x2t[:], AF.Square, accum_out=sqs[:, t : t + 1]
                )
                if t % 2 == 1:
                    # per-pair: mean = sum/512; var = sumsq/512 - mean^2;
                    # rstd = 1/sqrt(var+eps)
                    pr = slice(t - 1, t + 1)
                    nc.vector.tensor_scalar(
                        out=mean_v[:, pr], in0=sums[:, pr],
                        scalar1=1.0 / D, scalar2=None, op0=ALU.mult,
                    )
                    nc.vector.tensor_scalar(
                        out=q4[:, pr], in0=sqs[:, pr],
                        scalar1=1.0 / D, scalar2=None, op0=ALU.mult,
                    )
                    nc.vector.tensor_mul(m2[:, pr], mean_v[:, pr], mean_v[:, pr])
                    nc.vector.tensor_sub(var4[:, pr], q4[:, pr], m2[:, pr])
                    nc.scalar.activation(
                        sq42[:, pr], var4[:, pr], AF.Sqrt,
                        bias=eps_t[:], scale=1.0,
                    )
                    nc.vector.reciprocal(sq42[:, pr], sq42[:, pr])
            stage_b_mm(i + 1)
            stage_b_evict(i + 1)
            for _ in range(min(2, len(wload))):
                wload.pop()()
            # --- LN2 + transpose into y2T [128, kd2, i, 512] fp8 ---
            y2T = y2tp.tile([128, 2, 2, 512], F8, tag="y2T")
            for t in range(CB):
                norm_transp(
                    x2ts[t], mvs2, sq42, t, BF16, ident_bf,
                    y2T[:, :, :, t * 128 : (t + 1) * 128],
                    lambda p: p.rearrange("p (a c b) -> p a c b", a=2, c=2, b=128),
                )
            for _ in range(min(2, len(wload))):
                wload.pop()()
            # --- FFN1: fp8 DoubleRow, one PSUM bank per f-tile (two
            #     256-token groups), relu evict split ACT/DVE. Step i+2's
            #     LN1 chain is woven in mid-loop to fill idle engine slots ---
            h8 = hp.tile([128, NFT // 2, 2, 512], F8, tag="h")
            for ft in range(NFT):
                ph = psmm.tile([128, 512], F32, tag="mm")
                for hh in range(2):
                    for kd2 in range(2):
                        nc.tensor.matmul(
                            ph[:, hh * 256 : (hh + 1) * 256],
                            w18_r[:, kd2, :, ft * 128 : (ft + 1) * 128],
                            y2T[:, kd2, :, hh * 256 : (hh + 1) * 256],
                            start=(kd2 == 0), stop=(kd2 == 1),
                            perf_mode=DR,
                        )
                hdst = h8[:, ft // 2, ft % 2, :]
                hbc = hb_sb[:, ft : ft + 1]
                if ft in RELU_DVE:
                    nc.vector.tensor_scalar(
                        out=hdst, in0=ph[:], scalar1=hbc, scalar2=0.0,
                        op0=ALU.add, op1=ALU.max,
                    )
                else:
                    nc.scalar.activation(hdst, ph[:], AF.Relu, bias=hbc, scale=1.0)

            # --- FFN2: fp8 DoubleRow, two 256-col groups per out tile ---
            for t in range(CB):
                s0 = (c * CB + t) * T
                po = psmm.tile([128, D], F32, tag="mm")
                for eh in range(2):
                    for kt in range(NFT // 2):
                        nc.tensor.matmul(
                            po[:, eh * 256 : (eh + 1) * 256],
                            h8[:, kt, :, t * 128 : (t + 1) * 128],
                            w28_r[:, kt, :, eh * 256 : (eh + 1) * 256],
                            start=(kt == 0), stop=(kt == NFT // 2 - 1),
                            perf_mode=DR,
                        )
                ot = outp.tile([128, D], F32, tag="o")
                nc.vector.scalar_tensor_tensor(
                    out=ot[:], in0=po[:], scalar=OSC, in1=x2bts[t][:],
                    op0=ALU.mult, op1=ALU.add,
                )
                nc.sync.dma_start(out_d.ap()[s0 : s0 + T, b, :], ot[:])

    nc.compile()
    _NC_CACHE[key] = nc
    return nc


def _prep_inputs(x, w_lin, b_lin, w1, b1, w2, b2, g1, beta1, g2, beta2):
    f32 = np.float32
    wp = np.ascontiguousarray(w_lin.T * g1[:, None]).astype(NPBF16)
    zb = (w_lin.astype(np.float64) @ beta1.astype(np.float64) + b_lin).astype(
        f32
    ).astype(NPBF16)
    w1t8 = np.ascontiguousarray(W1SC * w1.T * g2[:, None]).astype(NPF8)
    hb16 = (W1SC * (w1.astype(np.float64) @ beta2.astype(np.float64) + b1)).astype(f32)
    w2t8 = np.ascontiguousarray(W2SC * w2.T).astype(NPF8)
    shared = {
        "wp": wp,
        "zb": zb,
        "w1t8": w1t8,
        "hb16": hb16,
        "w2t8": w2t8,
        "b2": b2.astype(f32),
        "wblk": _WBLKT.astype(NPBF16),
        "wlag": _WLAGT.astype(NPBF16),
    }
    in_maps = []
    for cc in range(NCORES):
        m = dict(shared)
        m["x"] = np.ascontiguousarray(x[:, cc * BL : (cc + 1) * BL, :]).astype(f32)
        in_maps.append(m)
    return in_maps


def kernel(**inputs):
    nc = build_nc()
    in_maps = _prep_inputs(**inputs)
    res = run_bass_kernel_spmd(nc, in_maps, list(range(NCORES)))
    out = np.concatenate([r["out"] for r in res.results], axis=1)
    return out.astype(np.float32)


if __name__ == "__main__":
    rng = np.random.default_rng(0)
    demo = {
        "x": rng.standard_normal((S, B, D)).astype(np.float32),
        "w_lin": rng.standard_normal((D, D)).astype(np.float32) * D**-0.5,
        "b_lin": rng.standard_normal((D,)).astype(np.float32) * 0.01,
        "w1": rng.standard_normal((FF, D)).astype(np.float32) * D**-0.5,
        "b1": rng.standard_normal((FF,)).astype(np.float32) * 0.01,
        "w2": rng.standard_normal((D, FF)).astype(np.float32) * FF**-0.5,
        "b2": rng.standard_normal((D,)).astype(np.float32) * 0.01,
        "g1": np.ones(D, np.float32),
        "beta1": np.zeros(D, np.float32),
        "g2": np.ones(D, np.float32),
        "beta2": np.zeros(D, np.float32),
    }
    out = kernel(**demo)
    print("ok", out.shape, out.dtype)


# revision 94
# speedup vs baseline: 1.0230x; 1.0230x over previous
"""Trainium2 Bass kernel for nn_ExpSelfAttention (dense transformer block).

Math (per batch item b):
    y  = LN(x; g1, beta1);  z = y @ w_lin.T + b_lin
    attn = W @ z            (W = causal exp-decay matrix, alpha=0.9)
    x2 = x + attn
    y2 = LN(x2; g2, beta2); h = relu(y2 @ w1.T + b1)
    out = x2 + h @ w2.T + b2

Sharding: data parallel over batch (16 / 8 cores = 2 per core); weights and
the (input-independent) decay-matrix blocks replicated. No collectives.

Kernel strategy per core (mixed precision):
  - Attention path in bf16 (proj + banded mixing matmuls, PE transposes at
    1 cyc/row); residuals and LN stats in f32. b_lin folded into the mixing
    PSUM via a K=1 ones-row bias matmul (W rows sum to 1, so W@(z+zb)=W@z+zb).
  - FFN matmuls in fp8e4 (e4m3) with DoubleRow perf mode: 256 contraction
    rows per instruction at 0.5 cyc per output element - 4x less PE time
    than f32r/bf16. Weights pre-scaled by 16 (w1) / 32 (w2) on the host to
    center fp8's [2^-6, 240] range; the 512x net factor is unwound in the
    output eviction (po * 1/512 + x2b).
  - The S x S decay matmul is block-banded (alpha^128 ~ 1.4e-6): exact
    block-diag + 1 lag matmul per 128-token block.
  - Engine balance: LN normalizes ((x-m)*rsqrt, subtract+mult - the Pool
    ucode has no divide) run on the otherwise-idle Pool (gpsimd) engine
    (SBUF-only ops - Pool has no PSUM port); PSUM evictions split between
    ACT and DVE; sqrt batched in pairs on ACT + DVE reciprocal. LN2
    transposes run in bf16 (hw fp8 transpose needs stride-2 outputs) and
    cast to fp8 at the ACT eviction.
  - FFN1 PSUM banks hold both 256-token half-chunk groups of one f-tile
    (sequential accumulation groups; PSUM data persists across a group
    start in the same bank), so each relu eviction covers [128,512].
  - Software pipelined in 512-token steps: step i+1's load/LN1/
    transpose/proj run between step i's mixing and FFN.
"""

import sys
from contextlib import ExitStack

for _p in ("/opt/trn_rl_repo", "/opt/pypackages"):
    if _p not in sys.path:
        sys.path.insert(0, _p)

import numpy as np
import ml_dtypes

import concourse.bass as bass
import concourse.mybir as mybir
import concourse.tile as tile
from concourse import bacc
from concourse.bass_utils import run_bass_kernel_spmd
from concourse.masks import make_identity

ALPHA, EPS = 0.9, 1e-5
S, B, D, FF = 2048, 16, 512, 2048
NCORES = 8
BL = B // NCORES            # batch items per core
T = 128                     # mixing block
CB = 4                      # blocks per chunk (step = 512 tokens)
NBLK = S // T               # 16
NCHUNK = NBLK // CB         # 4
NFT = FF // 128             # 16 f-tiles
KD = D // 128               # 4 d-tiles
NLAG = 1                    # decay lag blocks kept (lag>=2 < 2e-12 relative)
W1SC, W2SC = 16.0, 32.0     # fp8 weight pre-scales
OSC = 1.0 / (W1SC * W2SC)   # output unscale
RELU_DVE = frozenset({0, 2, 4, 6, 8, 10, 12, 14})  # relu evicts on DVE

F32 = mybir.dt.float32
BF16 = mybir.dt.bfloat16
F8 = mybir.dt.float8e4
AF = mybir.ActivationFunctionType
ALU = mybir.AluOpType
DR = mybir.MatmulPerfMode.DoubleRow

NPBF16 = ml_dtypes.bfloat16
NPF8 = mybir.dt.np(F8)      # ml_dtypes.float8_e4m3 (max 240)


def _host_consts():
    """Decay-matrix derived constants, f64 -> f32 (mirrors reference)."""
    i = np.arange(S, dtype=np.float64)
    diff = i[:, None] - i[None, :]
    with np.errstate(under="ignore"):
        W = np.where(diff >= 0, ALPHA ** (diff + 1), 0.0)
        W = W + np.diag(1.0 - W.sum(axis=1))
        W = W.astype(np.float32)
        blocks = [
            np.ascontiguousarray(W[c * T : (c + 1) * T, c * T : (c + 1) * T].T)
            for c in range(NBLK)
        ]
        uniq, idx = [], []
        for blk in blocks:
            for j, u in enumerate(uniq):
                if np.array_equal(blk, u):
                    idx.append(j)
                    break
            else:
                idx.append(len(uniq))
                uniq.append(blk)
        wblkT = np.stack(uniq)  # [NU, T, T]
        lags = []
        for l in range(1, NLAG + 1):
            L = W[l * T : (l + 1) * T, 0:T]
            lags.append(np.ascontiguousarray(L.T))
        wlagT = np.stack(lags)  # [NLAG, T, T]
    return wblkT.astype(np.float32), idx, wlagT.astype(np.float32)


_WBLKT, _BLKIDX, _WLAGT = _host_consts()
NU = _WBLKT.shape[0]

_NC_CACHE = {}


def build_nc():
    key = "v2"
    if key in _NC_CACHE:
        return _NC_CACHE[key]
    nc = bacc.Bacc()

    x_d = nc.declare_dram_parameter("x", [S, BL, D], F32, isOutput=False)
    wp_d = nc.declare_dram_parameter("wp", [D, D], BF16, isOutput=False)
    zb_d = nc.declare_dram_parameter("zb", [D], BF16, isOutput=False)
    w1_d = nc.declare_dram_parameter("w1t8", [D, FF], F8, isOutput=False)
    hb_d = nc.declare_dram_parameter("hb16", [FF], F32, isOutput=False)
    w2_d = nc.declare_dram_parameter("w2t8", [FF, D], F8, isOutput=False)
    b2_d = nc.declare_dram_parameter("b2", [D], F32, isOutput=False)
    wblk_d = nc.declare_dram_parameter("wblk", [NU, T, T], BF16, isOutput=False)
    wlag_d = nc.declare_dram_parameter("wlag", [NLAG, T, T], BF16, isOutput=False)
    out_d = nc.declare_dram_parameter("out", [S, BL, D], F32, isOutput=True)

    with tile.TileContext(nc) as tc, ExitStack() as ctx:
        pool = lambda name, bufs, **kw: ctx.enter_context(
            tc.tile_pool(name=name, bufs=bufs, **kw)
        )
        wgt = pool("wgt", 1)
        stage = pool("stage", 1)
        xin = pool("xin", 9)
        lnp = pool("ln", 4)
        yppp = pool("ypp", 7)
        xtp = pool("xt", 6)
        y2tp = pool("y2t", 2)
        zp = pool("z", 10)
        x2p = pool("x2", 5)
        x2bp = pool("x2b", 5)
        hp = pool("h", 2)
        outp = pool("outp", 3)
        psmm = pool("psmm", 5, space="PSUM")
        pstr = pool("pstr", 3, space="PSUM")

        # ---------------- one-time setup ----------------
        # Interleave the two batch items: consecutive steps are then fully
        # independent (the decay-lag z dependency is within one item), so
        # each step's LN chains overlap the neighbor step's FFN matmuls.
        steps = [(b, c) for c in range(NCHUNK) for b in range(BL)]
        xpre = {}

        def preload_x(i):
            if i >= len(steps) or i in xpre:
                return
            b, c = steps[i]
            tiles = []
            for t in range(CB):
                s0 = (c * CB + t) * T
                xt = xin.tile([128, D], F32, tag="x")
                nc.sync.dma_start(xt[:], x_d.ap()[s0 : s0 + T, b, :])
                tiles.append(xt)
            xpre[i] = tiles

        preload_x(0)
        preload_x(1)

        ident_f = stage.tile([128, 128], F32, tag="ident_f")
        make_identity(nc, ident_f[:])
        ident_bf = wgt.tile([128, 128], BF16, tag="ident_bf")
        nc.vector.tensor_copy(ident_bf[:], ident_f[:])
        ident_f8 = wgt.tile([128, 128], F8, tag="ident_f8")
        nc.vector.tensor_copy(ident_f8[:], ident_f[:])
        eps_t = wgt.tile([128, 1], F32, tag="eps")
        nc.vector.memset(eps_t[:], EPS)
        ones_r = wgt.tile([1, 128], BF16, tag="ones_r")
        nc.vector.memset(ones_r[:], 1.0)
        zb_r = wgt.tile([1, D], BF16, tag="zb_r")
        nc.sync.dma_start(zb_r[:], bass.AP(tensor=zb_d, offset=0, ap=[[0, 1], [1, D]]))
        b2_bc = wgt.tile([128, D], F32, tag="b2")
        nc.sync.dma_start(
            b2_bc[:], bass.AP(tensor=b2_d, offset=0, ap=[[0, 128], [1, D]])
        )
        hb_sb = wgt.tile([128, NFT], F32, tag="hb")
        nc.sync.dma_start(
            hb_sb[:], bass.AP(tensor=hb_d, offset=0, ap=[[1, 128], [128, NFT]])
        )
        wp_r = wgt.tile([128, KD, D], BF16, tag="wp")
        nc.sync.dma_start(wp_r[:], wp_d.ap().rearrange("(kd p) e -> p kd e", p=128))
        wblk_r = wgt.tile([128, NU, T], BF16, tag="wblk")
        nc.sync.dma_start(wblk_r[:], wblk_d.ap().rearrange("b j r -> j b r"))
        wlag_r = wgt.tile([128, NLAG, T], BF16, tag="wlag")
        nc.sync.dma_start(wlag_r[:], wlag_d.ap().rearrange("b j r -> j b r"))

        # ---------------- helpers ----------------
        def ln_stats(xts, tag):
            """4 tiles' LN stats -> (mvs [128,4,2], rs4 [128,4]=rsqrt(v+eps))."""
            mvs = lnp.tile([128, CB, 2], F32, tag=tag + "mv")
            rs4 = lnp.tile([128, CB], F32, tag=tag + "rs")
            for t in range(CB):
                st = lnp.tile([128, 6], F32, tag=tag + "st")
                nc.vector.bn_stats(st[:], xts[t][:])
                nc.vector.bn_aggr(mvs[:, t, :], st[:])
                if t % 2 == 1:
                    nc.scalar.activation(
                        rs4[:, t - 1 : t + 1],
                        mvs[:, t - 1 : t + 1, 1:2].rearrange("p a b -> p (a b)"),
                        AF.Sqrt, bias=eps_t[:], scale=1.0,
                    )
                    nc.vector.reciprocal(
                        rs4[:, t - 1 : t + 1], rs4[:, t - 1 : t + 1]
                    )
            return mvs, rs4

        def norm_transp(xt, mvs, sq4, t, odt, ident, dest_ap, src_pat):
            """(xt - m)/sq -> odt tile (Pool), PE-transpose, ACT-evict."""
            ypp = yppp.tile([128, D], odt, tag="ypp")
            nc.gpsimd.tensor_scalar(
                out=ypp[:], in0=xt[:],
                scalar1=mvs[:, t, 0:1], scalar2=sq4[:, t : t + 1],
                op0=ALU.subtract, op1=ALU.mult,
            )
            pt = pstr.tile([128, 512], odt, tag="tr")
            for kd in range(KD):
                nc.tensor.transpose(
                    pt[:, kd * 128 : (kd + 1) * 128],
                    ypp[:, kd * 128 : (kd + 1) * 128],
                    ident[:],
                )
            nc.scalar.activation(dest_ap, src_pat(pt[:]), AF.Copy)

        # ---------------- pipeline stages ----------------
        zall = {b: [] for b in range(BL)}
        a_out, b_out = {}, {}

        ypp_pre = {}

        def stage_a_pre(i):
            """LN1 stats + normalize for step i -> ypp tiles (bf16)."""
            if i >= len(steps):
                return
            preload_x(i)
            xts = xpre[i]
            mvs, sq4 = ln_stats(xts, "l1")
            ypps = []
            for t in range(CB):
                ypp = yppp.tile([128, D], BF16, tag="y1pp")
                nc.gpsimd.tensor_scalar(
                    out=ypp[:], in0=xts[t][:],
                    scalar1=mvs[:, t, 0:1], scalar2=sq4[:, t : t + 1],
                    op0=ALU.subtract, op1=ALU.mult,
                )
                ypps.append(ypp)
            ypp_pre[i] = ypps

        def stage_a_post(i):
            """PE transposes + ACT evicts of the normalized LN1 tiles."""
            if i >= len(steps):
                return
            xts, xT = xpre.pop(i), []
            for t, ypp in enumerate(ypp_pre.pop(i)):
                pt = pstr.tile([128, 512], BF16, tag="tr")
                for kd in range(KD):
                    nc.tensor.transpose(
                        pt[:, kd * 128 : (kd + 1) * 128],
                        ypp[:, kd * 128 : (kd + 1) * 128],
                        ident_bf[:],
                    )
                xTt = xtp.tile([128, KD, 128], BF16, tag="xT")
                nc.scalar.activation(
                    xTt[:], pt[:].rearrange("p (a b) -> p a b", b=128), AF.Copy
                )
                xT.append(xTt)
            a_out[i] = (xts, xT)

        def stage_b_mm(i):
            if i >= len(steps):
                return
            _, xT = a_out[i]
            pzs = []
            for t in range(CB):
                pz = psmm.tile([128, D], F32, tag="mm")
                for kd in range(KD):
                    nc.tensor.matmul(
                        pz[:], xT[t][:, kd, :], wp_r[:, kd, :],
                        start=(kd == 0), stop=(kd == KD - 1),
                    )
                pzs.append(pz)
            b_out[i] = pzs

        def stage_b_evict(i):
            if i >= len(steps):
                return
            b, c = steps[i]
            for t in range(CB):
                zt = zp.tile([128, D], BF16, tag="z")
                nc.scalar.activation(zt[:], b_out[i][t][:], AF.Copy)
                zall[b].append(zt)
            del b_out[i]

        stage_a_pre(0)
        stage_a_post(0)
        stage_b_mm(0)
        stage_b_evict(0)

        # big fp8 FFN weights: DMA'd in chunks interleaved with the early
        # pipeline so x loads and the first FFN aren't blocked.
        w18_r = wgt.tile([128, 2, 2, FF], F8, tag="w18")
        w28_r = wgt.tile([128, NFT // 2, 2, D], F8, tag="w28")
        w18_ap = w1_d.ap().rearrange("(kd2 i p) f -> p kd2 i f", p=128, i=2)
        w28_ap = w2_d.ap().rearrange("(kt i p) e -> p kt i e", p=128, i=2)
        wload = [
            lambda kd2=kd2: nc.sync.dma_start(
                w18_r[:, kd2, :, :], w18_ap[:, kd2, :, :]
            )
            for kd2 in range(2)
        ] + [
            lambda k4=k4: nc.sync.dma_start(
                w28_r[:, 2 * k4 : 2 * k4 + 2, :, :],
                w28_ap[:, 2 * k4 : 2 * k4 + 2, :, :],
            )
            for k4 in range(4)
        ]
        wload.reverse()
        for _ in range(2):
            wload.pop()()

        for i, (b, c) in enumerate(steps):
            xts, _ = a_out.pop(i)
            x2ts, x2bts, pms = [], [], []
            # --- mixing (banded) + zb ones-row bias matmul ---
            for t in range(CB):
                blk = c * CB + t
                nmix = 1 + min(blk, NLAG)
                pm = psmm.tile([128, D], F32, tag="mm")
                for l in range(nmix):
                    lhs = (
                        wblk_r[:, _BLKIDX[blk], :] if l == 0
                        else wlag_r[:, l - 1, :]
                    )
                    nc.tensor.matmul(
                        pm[:], lhs, zall[b][blk - l][:],
                        start=(l == 0), stop=False,
                    )
                nc.tensor.matmul(pm[:], ones_r[:], zb_r[:], start=False, stop=True)
                pms.append(pm)
            # --- next step's LN1 chain ---
            stage_a_pre(i + 1)
            stage_a_post(i + 1)
            for _ in range(min(2, len(wload))):
                wload.pop()()
            # --- x2 = pm + x (DVE, accumulating sum(x2) for free);
            # x2b = x2 + b2 (Pool); sum(x2^2) via ACT Square accumulator,
            # so LN2 statistics cost DVE only a few [128,4] ops ---
            sums = lnp.tile([128, CB], F32, tag="l2su")
            sqs = lnp.tile([128, CB], F32, tag="l2sq")
            mvs2 = lnp.tile([128, CB, 2], F32, tag="l2mv")
            q4 = lnp.tile([128, CB], F32, tag="l2q4")
            m2 = lnp.tile([128, CB], F32, tag="l2m2")
            var4 = lnp.tile([128, CB], F32, tag="l2v4")
            sq42 = lnp.tile([128, CB], F32, tag="l2rs")
            mean_v = mvs2[:, :, 0:1].rearrange("p a b -> p (a b)")
            for t in range(CB):
                x2t = x2p.tile([128, D], F32, tag="x2")
                nc.vector.scalar_tensor_tensor(
                    out=x2t[:], in0=pms[t][:], scalar=0.0, in1=xts[t][:],
                    op0=ALU.add, op1=ALU.add, accum_out=sums[:, t : t + 1],
                )
                x2ts.append(x2t)
                x2bt = x2bp.tile([128, D], F32, tag="x2b")
                nc.gpsimd.tensor_add(x2bt[:], x2t[:], b2_bc[:])
                x2bts.append(x2bt)
                scr = yppp.tile([128, D], BF16, tag="sqscr")
                nc.scalar.activation(
                    scr[:], x2t[:], AF.Square, accum_out=sqs[:, t : t + 1]
                )
                if t % 2 == 1:
                    # per-pair: mean = sum/512; var = sumsq/512 - mean^2;
                    # rstd = 1/sqrt(var+eps)
                    pr = slice(t - 1, t + 1)
                    nc.vector.tensor_scalar(
                        out=mean_v[:, pr], in0=sums[:, pr],
                        scalar1=1.0 / D, scalar2=None, op0=ALU.mult,
                    )
                    nc.vector.tensor_scalar(
                        out=q4[:, pr], in0=sqs[:, pr],
                        scalar1=1.0 / D, scalar2=None, op0=ALU.mult,
                    )
                    nc.vector.tensor_mul(m2[:, pr], mean_v[:, pr], mean_v[:, pr])
                    nc.vector.tensor_sub(var4[:, pr], q4[:, pr], m2[:, pr])
                    nc.scalar.activation(
                        sq42[:, pr], var4[:, pr], AF.Sqrt,
                        bias=eps_t[:], scale=1.0,
                    )
                    nc.vector.reciprocal(sq42[:, pr], sq42[:, pr])
            stage_b_mm(i + 1)
            stage_b_evict(i + 1)
            for _ in range(min(2, len(wload))):
                wload.pop()()
            # --- LN2 + transpose into y2T [128, kd2, i, 512] fp8 ---
            y2T = y2tp.tile([128, 2, 2, 512], F8, tag="y2T")
            for t in range(CB):
                norm_transp(
                    x2ts[t], mvs2, sq42, t, BF16, ident_bf,
                    y2T[:, :, :, t * 128 : (t + 1) * 128],
                    lambda p: p.rearrange("p (a c b) -> p a c b", a=2, c=2, b=128),
                )
            for _ in range(min(2, len(wload))):
                wload.pop()()
            # --- FFN1: fp8 DoubleRow, one PSUM bank per f-tile (two
            #     256-token groups), relu evict split ACT/DVE. Step i+2's
            #     LN1 chain is woven in mid-loop to fill idle engine slots ---
            h8 = hp.tile([128, NFT // 2, 2, 512], F8, tag="h")
            for ft in range(NFT):
                ph = psmm.tile([128, 512], F32, tag="mm")
                for hh in range(2):
                    for kd2 in range(2):
                        nc.tensor.matmul(
                            ph[:, hh * 256 : (hh + 1) * 256],
                            w18_r[:, kd2, :, ft * 128 : (ft + 1) * 128],
                            y2T[:, kd2, :, hh * 256 : (hh + 1) * 256],
                            start=(kd2 == 0), stop=(kd2 == 1),
                            perf_mode=DR,
                        )
                hdst = h8[:, ft // 2, ft % 2, :]
                hbc = hb_sb[:, ft : ft + 1]
                if ft in RELU_DVE:
                    nc.vector.tensor_scalar(
                        out=hdst, in0=ph[:], scalar1=hbc, scalar2=0.0,
                        op0=ALU.add, op1=ALU.max,
                    )
                else:
                    nc.scalar.activation(hdst, ph[:], AF.Relu, bias=hbc, scale=1.0)

            # --- FFN2: fp8 DoubleRow, two 256-col groups per out tile ---
            for t in range(CB):
                s0 = (c * CB + t) * T
                po = psmm.tile([128, D], F32, tag="mm")
                for eh in range(2):
                    for kt in range(NFT // 2):
                        nc.tensor.matmul(
                            po[:, eh * 256 : (eh + 1) * 256],
                            h8[:, kt, :, t * 128 : (t + 1) * 128],
                            w28_r[:, kt, :, eh * 256 : (eh + 1) * 256],
                            start=(kt == 0), stop=(kt == NFT // 2 - 1),
                            perf_mode=DR,
                        )
                ot = outp.tile([128, D], F32, tag="o")
                nc.vector.scalar_tensor_tensor(
                    out=ot[:], in0=po[:], scalar=OSC, in1=x2bts[t][:],
                    op0=ALU.mult, op1=ALU.add,
                )
                nc.sync.dma_start(out_d.ap()[s0 : s0 + T, b, :], ot[:])

    nc.compile()
    _NC_CACHE[key] = nc
    return nc


def _prep_inputs(x, w_lin, b_lin, w1, b1, w2, b2, g1, beta1, g2, beta2):
    f32 = np.float32
    wp = np.ascontiguousarray(w_lin.T * g1[:, None]).astype(NPBF16)
    zb = (w_lin.astype(np.float64) @ beta1.astype(np.float64) + b_lin).astype(
        f32
    ).astype(NPBF16)
    w1t8 = np.ascontiguousarray(W1SC * w1.T * g2[:, None]).astype(NPF8)
    hb16 = (W1SC * (w1.astype(np.float64) @ beta2.astype(np.float64) + b1)).astype(f32)
    w2t8 = np.ascontiguousarray(W2SC * w2.T).astype(NPF8)
    shared = {
        "wp": wp,
        "zb": zb,
        "w1t8": w1t8,
        "hb16": hb16,
        "w2t8": w2t8,
        "b2": b2.astype(f32),
        "wblk": _WBLKT.astype(NPBF16),
        "wlag": _WLAGT.astype(NPBF16),
    }
    in_maps = []
    for cc in range(NCORES):
        m = dict(shared)
        m["x"] = np.ascontiguousarray(x[:, cc * BL : (cc + 1) * BL, :]).astype(f32)
        in_maps.append(m)
    return in_maps


def kernel(**inputs):
    nc = build_nc()
    in_maps = _prep_inputs(**inputs)
    res = run_bass_kernel_spmd(nc, in_maps, list(range(NCORES)))
    out = np.concatenate([r["out"] for r in res.results], axis=1)
    return out.astype(np.float32)


if __name__ == "__main__":
    rng = np.random.default_rng(0)
    demo = {
        "x": rng.standard_normal((S, B, D)).astype(np.float32),
        "w_lin": rng.standard_normal((D, D)).astype(np.float32) * D**-0.5,
        "b_lin": rng.standard_normal((D,)).astype(np.float32) * 0.01,
        "w1": rng.standard_normal((FF, D)).astype(np.float32) * D**-0.5,
        "b1": rng.standard_normal((FF,)).astype(np.float32) * 0.01,
        "w2": rng.standard_normal((D, FF)).astype(np.float32) * FF**-0.5,
        "b2": rng.standard_normal((D,)).astype(np.float32) * 0.01,
        "g1": np.ones(D, np.float32),
        "beta1": np.zeros(D, np.float32),
        "g2": np.ones(D, np.float32),
        "beta2": np.zeros(D, np.float32),
    }
    out = kernel(**demo)
    print("ok", out.shape, out.dtype)


# revision 95
# speedup vs baseline: 1.0305x; 1.0073x over previous
"""Trainium2 Bass kernel for nn_ExpSelfAttention (dense transformer block).

Math (per batch item b):
    y  = LN(x; g1, beta1);  z = y @ w_lin.T + b_lin
    attn = W @ z            (W = causal exp-decay matrix, alpha=0.9)
    x2 = x + attn
    y2 = LN(x2; g2, beta2); h = relu(y2 @ w1.T + b1)
    out = x2 + h @ w2.T + b2

Sharding: data parallel over batch (16 / 8 cores = 2 per core); weights and
the (input-independent) decay-matrix blocks replicated. No collectives.

Kernel strategy per core (mixed precision):
  - Attention path in bf16 (proj + banded mixing matmuls, PE transposes at
    1 cyc/row); residuals and LN stats in f32. b_lin folded into the mixing
    PSUM via a K=1 ones-row bias matmul (W rows sum to 1, so W@(z+zb)=W@z+zb).
  - FFN matmuls in fp8e4 (e4m3) with DoubleRow perf mode: 256 contraction
    rows per instruction at 0.5 cyc per output element - 4x less PE time
    than f32r/bf16. Weights pre-scaled by 16 (w1) / 32 (w2) on the host to
    center fp8's [2^-6, 240] range; the 512x net factor is unwound in the
    output eviction (po * 1/512 + x2b).
  - The S x S decay matmul is block-banded (alpha^128 ~ 1.4e-6): exact
    block-diag + 1 lag matmul per 128-token block.
  - Engine balance: LN normalizes ((x-m)*rsqrt, subtract+mult - the Pool
    ucode has no divide) run on the otherwise-idle Pool (gpsimd) engine
    (SBUF-only ops - Pool has no PSUM port); PSUM evictions split between
    ACT and DVE; sqrt batched in pairs on ACT + DVE reciprocal. LN2
    transposes run in bf16 (hw fp8 transpose needs stride-2 outputs) and
    cast to fp8 at the ACT eviction.
  - FFN1 PSUM banks hold both 256-token half-chunk groups of one f-tile
    (sequential accumulation groups; PSUM data persists across a group
    start in the same bank), so each relu eviction covers [128,512].
  - Software pipelined in 512-token steps: step i+1's load/LN1/
    transpose/proj run between step i's mixing and FFN.
"""

import sys
from contextlib import ExitStack

for _p in ("/opt/trn_rl_repo", "/opt/pypackages"):
    if _p not in sys.path:
        sys.path.insert(0, _p)

import numpy as np
import ml_dtypes

import concourse.bass as bass
import concourse.mybir as mybir
import concourse.tile as tile
from concourse import bacc
from concourse.bass_utils import run_bass_kernel_spmd
from concourse.masks import make_identity

ALPHA, EPS = 0.9, 1e-5
S, B, D, FF = 2048, 16, 512, 2048
NCORES = 8
BL = B // NCORES            # batch items per core
T = 128                     # mixing block
CB = 4                      # blocks per chunk (step = 512 tokens)
NBLK = S // T               # 16
NCHUNK = NBLK // CB         # 4
NFT = FF // 128             # 16 f-tiles
KD = D // 128               # 4 d-tiles
NLAG = 1                    # decay lag blocks kept (lag>=2 < 2e-12 relative)
W1SC, W2SC = 16.0, 32.0     # fp8 weight pre-scales
OSC = 1.0 / (W1SC * W2SC)   # output unscale
RELU_DVE = frozenset({1, 3, 5, 7, 9, 11, 13, 15})  # relu evicts on DVE

F32 = mybir.dt.float32
BF16 = mybir.dt.bfloat16
F8 = mybir.dt.float8e4
AF = mybir.ActivationFunctionType
ALU = mybir.AluOpType
DR = mybir.MatmulPerfMode.DoubleRow

NPBF16 = ml_dtypes.bfloat16
NPF8 = mybir.dt.np(F8)      # ml_dtypes.float8_e4m3 (max 240)


def _host_consts():
    """Decay-matrix derived constants, f64 -> f32 (mirrors reference)."""
    i = np.arange(S, dtype=np.float64)
    diff = i[:, None] - i[None, :]
    with np.errstate(under="ignore"):
        W = np.where(diff >= 0, ALPHA ** (diff + 1), 0.0)
        W = W + np.diag(1.0 - W.sum(axis=1))
        W = W.astype(np.float32)
        blocks = [
            np.ascontiguousarray(W[c * T : (c + 1) * T, c * T : (c + 1) * T].T)
            for c in range(NBLK)
        ]
        uniq, idx = [], []
        for blk in blocks:
            for j, u in enumerate(uniq):
                if np.array_equal(blk, u):
                    idx.append(j)
                    break
            else:
                idx.append(len(uniq))
                uniq.append(blk)
        wblkT = np.stack(uniq)  # [NU, T, T]
        lags = []
        for l in range(1, NLAG + 1):
            L = W[l * T : (l + 1) * T, 0:T]
            lags.append(np.ascontiguousarray(L.T))
        wlagT = np.stack(lags)  # [NLAG, T, T]
    return wblkT.astype(np.float32), idx, wlagT.astype(np.float32)


_WBLKT, _BLKIDX, _WLAGT = _host_consts()
NU = _WBLKT.shape[0]

_NC_CACHE = {}


def build_nc():
    key = "v2"
    if key in _NC_CACHE:
        return _NC_CACHE[key]
    nc = bacc.Bacc()

    x_d = nc.declare_dram_parameter("x", [S, BL, D], F32, isOutput=False)
    wp_d = nc.declare_dram_parameter("wp", [D, D], BF16, isOutput=False)
    zb_d = nc.declare_dram_parameter("zb", [D], BF16, isOutput=False)
    w1_d = nc.declare_dram_parameter("w1t8", [D, FF], F8, isOutput=False)
    hb_d = nc.declare_dram_parameter("hb16", [FF], F32, isOutput=False)
    w2_d = nc.declare_dram_parameter("w2t8", [FF, D], F8, isOutput=False)
    b2_d = nc.declare_dram_parameter("b2", [D], F32, isOutput=False)
    wblk_d = nc.declare_dram_parameter("wblk", [NU, T, T], BF16, isOutput=False)
    wlag_d = nc.declare_dram_parameter("wlag", [NLAG, T, T], BF16, isOutput=False)
    out_d = nc.declare_dram_parameter("out", [S, BL, D], F32, isOutput=True)

    with tile.TileContext(nc) as tc, ExitStack() as ctx:
        pool = lambda name, bufs, **kw: ctx.enter_context(
            tc.tile_pool(name=name, bufs=bufs, **kw)
        )
        wgt = pool("wgt", 1)
        stage = pool("stage", 1)
        xin = pool("xin", 9)
        lnp = pool("ln", 4)
        yppp = pool("ypp", 7)
        xtp = pool("xt", 6)
        y2tp = pool("y2t", 2)
        zp = pool("z", 10)
        x2p = pool("x2", 5)
        x2bp = pool("x2b", 5)
        hp = pool("h", 2)
        outp = pool("outp", 3)
        psmm = pool("psmm", 5, space="PSUM")
        pstr = pool("pstr", 3, space="PSUM")

        # ---------------- one-time setup ----------------
        # Interleave the two batch items: consecutive steps are then fully
        # independent (the decay-lag z dependency is within one item), so
        # each step's LN chains overlap the neighbor step's FFN matmuls.
        steps = [(b, c) for c in range(NCHUNK) for b in range(BL)]
        xpre = {}

        def preload_x(i):
            if i >= len(steps) or i in xpre:
                return
            b, c = steps[i]
            tiles = []
            for t in range(CB):
                s0 = (c * CB + t) * T
                xt = xin.tile([128, D], F32, tag="x")
                nc.sync.dma_start(xt[:], x_d.ap()[s0 : s0 + T, b, :])
                tiles.append(xt)
            xpre[i] = tiles

        preload_x(0)
        preload_x(1)

        ident_f = stage.tile([128, 128], F32, tag="ident_f")
        make_identity(nc, ident_f[:])
        ident_bf = wgt.tile([128, 128], BF16, tag="ident_bf")
        nc.vector.tensor_copy(ident_bf[:], ident_f[:])
        ident_f8 = wgt.tile([128, 128], F8, tag="ident_f8")
        nc.vector.tensor_copy(ident_f8[:], ident_f[:])
        eps_t = wgt.tile([128, 1], F32, tag="eps")
        nc.vector.memset(eps_t[:], EPS)
        ones_r = wgt.tile([1, 128], BF16, tag="ones_r")
        nc.vector.memset(ones_r[:], 1.0)
        zb_r = wgt.tile([1, D], BF16, tag="zb_r")
        nc.sync.dma_start(zb_r[:], bass.AP(tensor=zb_d, offset=0, ap=[[0, 1], [1, D]]))
        b2_bc = wgt.tile([128, D], F32, tag="b2")
        nc.sync.dma_start(
            b2_bc[:], bass.AP(tensor=b2_d, offset=0, ap=[[0, 128], [1, D]])
        )
        hb_sb = wgt.tile([128, NFT], F32, tag="hb")
        nc.sync.dma_start(
            hb_sb[:], bass.AP(tensor=hb_d, offset=0, ap=[[1, 128], [128, NFT]])
        )
        wp_r = wgt.tile([128, KD, D], BF16, tag="wp")
        nc.sync.dma_start(wp_r[:], wp_d.ap().rearrange("(kd p) e -> p kd e", p=128))
        wblk_r = wgt.tile([128, NU, T], BF16, tag="wblk")
        nc.sync.dma_start(wblk_r[:], wblk_d.ap().rearrange("b j r -> j b r"))
        wlag_r = wgt.tile([128, NLAG, T], BF16, tag="wlag")
        nc.sync.dma_start(wlag_r[:], wlag_d.ap().rearrange("b j r -> j b r"))

        # ---------------- helpers ----------------
        def ln_stats(xts, tag):
            """4 tiles' LN stats -> (mvs [128,4,2], rs4 [128,4]=rsqrt(v+eps))."""
            mvs = lnp.tile([128, CB, 2], F32, tag=tag + "mv")
            rs4 = lnp.tile([128, CB], F32, tag=tag + "rs")
            for t in range(CB):
                st = lnp.tile([128, 6], F32, tag=tag + "st")
                nc.vector.bn_stats(st[:], xts[t][:])
                nc.vector.bn_aggr(mvs[:, t, :], st[:])
                if t % 2 == 1:
                    nc.scalar.activation(
                        rs4[:, t - 1 : t + 1],
                        mvs[:, t - 1 : t + 1, 1:2].rearrange("p a b -> p (a b)"),
                        AF.Sqrt, bias=eps_t[:], scale=1.0,
                    )
                    nc.vector.reciprocal(
                        rs4[:, t - 1 : t + 1], rs4[:, t - 1 : t + 1]
                    )
            return mvs, rs4

        def norm_transp(xt, mvs, sq4, t, odt, ident, dest_ap, src_pat):
            """(xt - m)/sq -> odt tile (Pool), PE-transpose, ACT-evict."""
            ypp = yppp.tile([128, D], odt, tag="ypp")
            nc.gpsimd.tensor_scalar(
                out=ypp[:], in0=xt[:],
                scalar1=mvs[:, t, 0:1], scalar2=sq4[:, t : t + 1],
                op0=ALU.subtract, op1=ALU.mult,
            )
            pt = pstr.tile([128, 512], odt, tag="tr")
            for kd in range(KD):
                nc.tensor.transpose(
                    pt[:, kd * 128 : (kd + 1) * 128],
                    ypp[:, kd * 128 : (kd + 1) * 128],
                    ident[:],
                )
            nc.scalar.activation(dest_ap, src_pat(pt[:]), AF.Copy)

        # ---------------- pipeline stages ----------------
        zall = {b: [] for b in range(BL)}
        a_out, b_out = {}, {}

        ypp_pre = {}

        def stage_a_pre(i):
            """LN1 stats + normalize for step i -> ypp tiles (bf16)."""
            if i >= len(steps):
                return
            preload_x(i)
            xts = xpre[i]
            mvs, sq4 = ln_stats(xts, "l1")
            ypps = []
            for t in range(CB):
                ypp = yppp.tile([128, D], BF16, tag="y1pp")
                nc.gpsimd.tensor_scalar(
                    out=ypp[:], in0=xts[t][:],
                    scalar1=mvs[:, t, 0:1], scalar2=sq4[:, t : t + 1],
                    op0=ALU.subtract, op1=ALU.mult,
                )
                ypps.append(ypp)
            ypp_pre[i] = ypps

        def stage_a_post(i):
            """PE transposes + ACT evicts of the normalized LN1 tiles."""
            if i >= len(steps):
                return
            xts, xT = xpre.pop(i), []
            for t, ypp in enumerate(ypp_pre.pop(i)):
                pt = pstr.tile([128, 512], BF16, tag="tr")
                for kd in range(KD):
                    nc.tensor.transpose(
                        pt[:, kd * 128 : (kd + 1) * 128],
                        ypp[:, kd * 128 : (kd + 1) * 128],
                        ident_bf[:],
                    )
                xTt = xtp.tile([128, KD, 128], BF16, tag="xT")
                nc.scalar.activation(
                    xTt[:], pt[:].rearrange("p (a b) -> p a b", b=128), AF.Copy
                )
                xT.append(xTt)
            a_out[i] = (xts, xT)

        def stage_b_mm(i):
            if i >= len(steps):
                return
            _, xT = a_out[i]
            pzs = []
            for t in range(CB):
                pz = psmm.tile([128, D], F32, tag="mm")
                for kd in range(KD):
                    nc.tensor.matmul(
                        pz[:], xT[t][:, kd, :], wp_r[:, kd, :],
                        start=(kd == 0), stop=(kd == KD - 1),
                    )
                pzs.append(pz)
            b_out[i] = pzs

        def stage_b_evict(i):
            if i >= len(steps):
                return
            b, c = steps[i]
            for t in range(CB):
                zt = zp.tile([128, D], BF16, tag="z")
                nc.scalar.activation(zt[:], b_out[i][t][:], AF.Copy)
                zall[b].append(zt)
            del b_out[i]

        stage_a_pre(0)
        stage_a_post(0)
        stage_b_mm(0)
        stage_b_evict(0)

        # big fp8 FFN weights: DMA'd in chunks interleaved with the early
        # pipeline so x loads and the first FFN aren't blocked.
        w18_r = wgt.tile([128, 2, 2, FF], F8, tag="w18")
        w28_r = wgt.tile([128, NFT // 2, 2, D], F8, tag="w28")
        w18_ap = w1_d.ap().rearrange("(kd2 i p) f -> p kd2 i f", p=128, i=2)
        w28_ap = w2_d.ap().rearrange("(kt i p) e -> p kt i e", p=128, i=2)
        wload = [
            lambda kd2=kd2: nc.sync.dma_start(
                w18_r[:, kd2, :, :], w18_ap[:, kd2, :, :]
            )
            for kd2 in range(2)
        ] + [
            lambda k4=k4: nc.sync.dma_start(
                w28_r[:, 2 * k4 : 2 * k4 + 2, :, :],
                w28_ap[:, 2 * k4 : 2 * k4 + 2, :, :],
            )
            for k4 in range(4)
        ]
        wload.reverse()
        for _ in range(2):
            wload.pop()()

        for i, (b, c) in enumerate(steps):
            xts, _ = a_out.pop(i)
            x2ts, x2bts, pms = [], [], []
            # --- mixing (banded) + zb ones-row bias matmul ---
            for t in range(CB):
                blk = c * CB + t
                nmix = 1 + min(blk, NLAG)
                pm = psmm.tile([128, D], F32, tag="mm")
                for l in range(nmix):
                    lhs = (
                        wblk_r[:, _BLKIDX[blk], :] if l == 0
                        else wlag_r[:, l - 1, :]
                    )
                    nc.tensor.matmul(
                        pm[:], lhs, zall[b][blk - l][:],
                        start=(l == 0), stop=False,
                    )
                nc.tensor.matmul(pm[:], ones_r[:], zb_r[:], start=False, stop=True)
                pms.append(pm)
            # --- next step's LN1 chain ---
            stage_a_pre(i + 1)
            stage_a_post(i + 1)
            for _ in range(min(2, len(wload))):
                wload.pop()()
            # --- x2 = pm + x (DVE, accumulating sum(x2) for free);
            # x2b = x2 + b2 (Pool); sum(x2^2) via ACT Square accumulator,
            # so LN2 statistics cost DVE only a few [128,4] ops ---
            sums = lnp.tile([128, CB], F32, tag="l2su")
            sqs = lnp.tile([128, CB], F32, tag="l2sq")
            mvs2 = lnp.tile([128, CB, 2], F32, tag="l2mv")
            q4 = lnp.tile([128, CB], F32, tag="l2q4")
            m2 = lnp.tile([128, CB], F32, tag="l2m2")
            var4 = lnp.tile([128, CB], F32, tag="l2v4")
            sq42 = lnp.tile([128, CB], F32, tag="l2rs")
            mean_v = mvs2[:, :, 0:1].rearrange("p a b -> p (a b)")
            for t in range(CB):
                x2t = x2p.tile([128, D], F32, tag="x2")
                nc.vector.scalar_tensor_tensor(
                    out=x2t[:], in0=pms[t][:], scalar=0.0, in1=xts[t][:],
                    op0=ALU.add, op1=ALU.add, accum_out=sums[:, t : t + 1],
                )
                x2ts.append(x2t)
                x2bt = x2bp.tile([128, D], F32, tag="x2b")
                nc.gpsimd.tensor_add(x2bt[:], x2t[:], b2_bc[:])
                x2bts.append(x2bt)
                scr = yppp.tile([128, D], BF16, tag="sqscr")
                nc.scalar.activation(
                    scr[:], x2t[:], AF.Square, accum_out=sqs[:, t : t + 1]
                )
                if t % 2 == 1:
                    # per-pair: mean = sum/512; var = sumsq/512 - mean^2;
                    # rstd = 1/sqrt(var+eps)
                    pr = slice(t - 1, t + 1)
                    nc.vector.tensor_scalar(
                        out=mean_v[:, pr], in0=sums[:, pr],
                        scalar1=1.0 / D, scalar2=None, op0=ALU.mult,
                    )
                    nc.vector.tensor_scalar(
                        out=q4[:, pr], in0=sqs[:, pr],
                        scalar1=1.0 / D, scalar2=None, op0=ALU.mult,
                    )
                    nc.vector.tensor_mul(m2[:, pr], mean_v[:, pr], mean_v[:, pr])
                    nc.vector.tensor_sub(var4[:, pr], q4[:, pr], m2[:, pr])
                    nc.scalar.activation(
                        sq42[:, pr], var4[:, pr], AF.Sqrt,
                        bias=eps_t[:], scale=1.0,
                    )
                    nc.vector.reciprocal(sq42[:, pr], sq42[:, pr])
            stage_b_mm(i + 1)
            stage_b_evict(i + 1)
            for _ in range(min(2, len(wload))):
                wload.pop()()
            # --- LN2 + transpose into y2T [128, kd2, i, 512] fp8 ---
            y2T = y2tp.tile([128, 2, 2, 512], F8, tag="y2T")
            for t in range(CB):
                norm_transp(
                    x2ts[t], mvs2, sq42, t, BF16, ident_bf,
                    y2T[:, :, :, t * 128 : (t + 1) * 128],
                    lambda p: p.rearrange("p (a c b) -> p a c b", a=2, c=2, b=128),
                )
            for _ in range(min(2, len(wload))):
                wload.pop()()
            # --- FFN1: fp8 DoubleRow, one PSUM bank per f-tile (two
            #     256-token groups), relu evict split ACT/DVE. Step i+2's
            #     LN1 chain is woven in mid-loop to fill idle engine slots ---
            h8 = hp.tile([128, NFT // 2, 2, 512], F8, tag="h")
            for ft in range(NFT):
                ph = psmm.tile([128, 512], F32, tag="mm")
                for hh in range(2):
                    for kd2 in range(2):
                        nc.tensor.matmul(
                            ph[:, hh * 256 : (hh + 1) * 256],
                            w18_r[:, kd2, :, ft * 128 : (ft + 1) * 128],
                            y2T[:, kd2, :, hh * 256 : (hh + 1) * 256],
                            start=(kd2 == 0), stop=(kd2 == 1),
                            perf_mode=DR,
                        )
                hdst = h8[:, ft // 2, ft % 2, :]
                hbc = hb_sb[:, ft : ft + 1]
                if ft in RELU_DVE:
                    nc.vector.tensor_scalar(
                        out=hdst, in0=ph[:], scalar1=hbc, scalar2=0.0,
                        op0=ALU.add, op1=ALU.max,
                    )
                else:
                    nc.scalar.activation(hdst, ph[:], AF.Relu, bias=hbc, scale=1.0)

            # --- FFN2: fp8 DoubleRow, two 256-col groups per out tile ---
            for t in range(CB):
                s0 = (c * CB + t) * T
                po = psmm.tile([128, D], F32, tag="mm")
                for eh in range(2):
                    for kt in range(NFT // 2):
                        nc.tensor.matmul(
                            po[:, eh * 256 : (eh + 1) * 256],
                            h8[:, kt, :, t * 128 : (t + 1) * 128],
                            w28_r[:, kt, :, eh * 256 : (eh + 1) * 256],
                            start=(kt == 0), stop=(kt == NFT // 2 - 1),
                            perf_mode=DR,
                        )
                ot = outp.tile([128, D], F32, tag="o")
                nc.vector.scalar_tensor_tensor(
                    out=ot[:], in0=po[:], scalar=OSC, in1=x2bts[t][:],
                    op0=ALU.mult, op1=ALU.add,
                )
                nc.sync.dma_start(out_d.ap()[s0 : s0 + T, b, :], ot[:])

    nc.compile()
    _NC_CACHE[key] = nc
    return nc


def _prep_inputs(x, w_lin, b_lin, w1, b1, w2, b2, g1, beta1, g2, beta2):
    f32 = np.float32
    wp = np.ascontiguousarray(w_lin.T * g1[:, None]).astype(NPBF16)
    zb = (w_lin.astype(np.float64) @ beta1.astype(np.float64) + b_lin).astype(
        f32
    ).astype(NPBF16)
    w1t8 = np.ascontiguousarray(W1SC * w1.T * g2[:, None]).astype(NPF8)
    hb16 = (W1SC * (w1.astype(np.float64) @ beta2.astype(np.float64) + b1)).astype(f32)
    w2t8 = np.ascontiguousarray(W2SC * w2.T).astype(NPF8)
    shared = {
        "wp": wp,
        "zb": zb,
        "w1t8": w1t8,
        "hb16": hb16,
        "w2t8": w2t8,
        "b2": b2.astype(f32),
        "wblk": _WBLKT.astype(NPBF16),
        "wlag": _WLAGT.astype(NPBF16),
    }
    in_maps = []
    for cc in range(NCORES):
        m = dict(shared)
        m["x"] = np.ascontiguousarray(x[:, cc * BL : (cc + 1) * BL, :]).astype(f32)
        in_maps.append(m)
    return in_maps


def kernel(**inputs):
    nc = build_nc()
    in_maps = _prep_inputs(**inputs)
    res = run_bass_kernel_spmd(nc, in_maps, list(range(NCORES)))
    out = np.concatenate([r["out"] for r in res.results], axis=1)
    return out.astype(np.float32)


if __name__ == "__main__":
    rng = np.random.default_rng(0)
    demo = {
        "x": rng.standard_normal((S, B, D)).astype(np.float32),
        "w_lin": rng.standard_normal((D, D)).astype(np.float32) * D**-0.5,
        "b_lin": rng.standard_normal((D,)).astype(np.float32) * 0.01,
        "w1": rng.standard_normal((FF, D)).astype(np.float32) * D**-0.5,
        "b1": rng.standard_normal((FF,)).astype(np.float32) * 0.01,
        "w2": rng.standard_normal((D, FF)).astype(np.float32) * FF**-0.5,
        "b2": rng.standard_normal((D,)).astype(np.float32) * 0.01,
        "g1": np.ones(D, np.float32),
        "beta1": np.zeros(D, np.float32),
        "g2": np.ones(D, np.float32),
        "beta2": np.zeros(D, np.float32),
    }
    out = kernel(**demo)
    print("ok", out.shape, out.dtype)
